# revision 1
# baseline (speedup 1.0000x reference)
"""Trainium2 Bass kernel for nn_CorrTorch: 27-shift 3D correlation + 1x1x1 conv.

Math (B=1, C=32, D=H=W=64, NOFF=27):
  cv[(k,c), s] = x1[c,s] * pad(x2)[c, s + off_k] / sqrt(C)    (864 x 64^3)
  out[o, s]    = sum_{k,c} conv_w[o, k*32+c] * cv[(k,c), s] + conv_b[o]

Sharding: D axis split across 8 cores (8 planes each), 1-voxel halo baked into
the per-core x2 slab on the host. No collectives.

Per-core device strategy:
  - 3 "replica groups" of 32 channels on partitions 0..95; group g holds data
    pre-shifted by dx=g along W (host-baked), so all 27 shifts reduce to
    9 product passes (dz,dy in {0,1,2}^2), each a single bf16 tensor_tensor
    multiply in 2x mode over 96 partitions (9 passes is provably minimal for
    any fixed-preshift replica scheme).  ~2 of the 9 passes per plane run on
    GPSIMD, in parallel with the vector engine (the bottleneck).
  - The 1x1 conv becomes 9 accumulated matmuls (K=96 contraction chunks) with
    M=27 output channels.  M<32, so 4 spatial subtiles are processed
    concurrently via tile_position column-tiling (col group g4 -> psum
    partitions 32*g4..32*g4+26).
  - ScalarE evicts PSUM -> SBUF fp32 with the conv bias applied per-partition,
    then HWDGE DMA writes straight to HBM.
"""

import numpy as np
import ml_dtypes

import concourse.bass as bass
import concourse.mybir as mybir
import concourse.tile as tile
from concourse.bass_utils import run_bass_kernel_spmd

C = 32
D = 64
H = 64
W = 64
NOFF = 27
NCORES = 8
DLOC = D // NCORES          # 8 output planes per core
NSLAB = DLOC + 2            # 10 padded x2 planes per core
HP = H + 2                  # 66
WP = W + 2                  # 66
PLANE_F = HP * WP           # 4356 elements per padded plane per partition
G = 3                       # dx replica groups
P96 = G * C                 # 96 partitions used by products / contraction
NPASS = 9                   # (dz, dy) passes
TN = H * W                  # 4096 columns per cv tile (one full plane)
SUB = 512                   # columns per matmul (one PSUM bank)
NSUBT = TN // SUB           # 8 spatial subtiles per plane
NSUB = 4                    # col-tiled concurrent matmul groups

BF16 = mybir.dt.bfloat16
F32 = mybir.dt.float32

_wsplit_ctr = [0]


def _split_sync_waits(nc, max_waits=1):
    """Walrus in this container accepts at most one sync wait per instruction.
    Hoist excess waits onto NoOp instructions inserted just before, on the
    same engine (same-engine program order preserves the semantics)."""
    for fn in nc.m.functions:
        for bb in fn.blocks:
            new = []
            changed = False
            for ins in bb.instructions:
                si = ins.sync_info
                if si is not None and len(si.on_wait) > max_waits:
                    waits = list(si.on_wait)
                    excess, keep = waits[:-max_waits], waits[-max_waits:]
                    for i in range(0, len(excess), max_waits):
                        _wsplit_ctr[0] += 1
                        new.append(
                            mybir.InstNoOp(
                                name=f"wsplit-{_wsplit_ctr[0]}",
                                engine=ins.engine,
                                sync_info=mybir.SyncInfo(
                                    on_wait=excess[i : i + max_waits], on_update=[]
                                ),
                            )
                        )
                    ins.sync_info = mybir.SyncInfo(
                        on_wait=keep, on_update=list(si.on_update)
                    )
                    changed = True
                new.append(ins)
            if changed:
                bb.instructions = new


def build_program():
    nc = bass.Bass()

    x1r = nc.dram_tensor("x1r", [DLOC, P96, H * W], BF16, kind="ExternalInput")
    x2r = nc.dram_tensor("x2r", [NSLAB, P96, PLANE_F], BF16, kind="ExternalInput")
    wts = nc.dram_tensor("wts", [P96, NPASS * NOFF], BF16, kind="ExternalInput")
    bias = nc.dram_tensor("bias", [128, 1], F32, kind="ExternalInput")
    out = nc.dram_tensor("out", [NOFF, DLOC * H * W], F32, kind="ExternalOutput")

    with tile.TileContext(nc) as tc:
        with (
            tc.tile_pool(name="wt", bufs=1) as wt_pool,
            tc.tile_pool(name="x2", bufs=5) as x2_pool,
            tc.tile_pool(name="x1", bufs=3) as x1_pool,
            tc.tile_pool(name="cv", bufs=11) as cv_pool,
            tc.tile_pool(name="stage", bufs=3) as stage_pool,
            tc.tile_pool(name="psum", bufs=4, space="PSUM") as psum_pool,
        ):
            x2t = {}

            def load_x2_plane(p):
                t = x2_pool.tile([P96, HP, WP], BF16, tag="x2plane")
                nc.sync.dma_start(out=t[:], in_=x2r[p])
                x2t[p] = t

            # first compute pass needs only x1[0] + x2[0]; issue those first
            x1t0 = x1_pool.tile([P96, H * W], BF16, tag="x1plane")
            nc.sync.dma_start(out=x1t0[:], in_=x1r[0])
            load_x2_plane(0)
            wt_tile = wt_pool.tile([P96, NPASS * NOFF], BF16)
            nc.sync.dma_start(out=wt_tile[:], in_=wts[:])
            bias_tile = wt_pool.tile([128, 1], F32)
            nc.sync.dma_start(out=bias_tile[:], in_=bias[:])
            for p in range(1, 3):
                load_x2_plane(p)

            for d in range(DLOC):
                if d + 3 < NSLAB:
                    load_x2_plane(d + 3)
                if d == 0:
                    x1t = x1t0
                else:
                    x1t = x1_pool.tile([P96, H * W], BF16, tag="x1plane")
                    nc.sync.dma_start(out=x1t[:], in_=x1r[d])

                psums = []
                for _ph in range(2):
                    ps = psum_pool.tile([128, SUB], F32, tag="ps")
                    psums.append(ps)
                for dz in range(3):
                    for dy in range(3):
                        j = 3 * dz + dy
                        cv = cv_pool.tile([P96, TN], BF16, tag="cv")
                        # ~2 passes per plane run on GPSIMD in parallel with
                        # the rest on the (bottleneck) vector engine
                        offload = (d < 7 and (dz, dy) in ((0, 2), (2, 2))) or (
                            d == 7 and (dz, dy) == (2, 2)
                        )
                        eng = nc.gpsimd if offload else nc.vector
                        eng.tensor_mul(
                            out=cv[:],
                            in0=x1t[:],
                            in1=x2t[d + dz][:, dy : dy + H, 0:W],
                        )
                        for s in range(NSUBT):
                            nc.tensor.matmul(
                                psums[s // NSUB][32 * (s % NSUB) : 32 * (s % NSUB) + NOFF, :],
                                lhsT=wt_tile[:, j * NOFF : (j + 1) * NOFF],
                                rhs=cv[:, s * SUB : (s + 1) * SUB],
                                start=(j == 0),
                                stop=(j == NPASS - 1),
                                tile_position=(0, 32 * (s % NSUB)),
                            )
                for half in range(2):
                    stage = stage_pool.tile([128, SUB], F32, tag="stage")
                    nc.scalar.activation(
                        stage[:],
                        psums[half][:],
                        mybir.ActivationFunctionType.Identity,
                        bias=bias_tile[:],
                    )
                    base = d * (H * W) + half * (TN // 2)
                    for g4 in range(NSUB):
                        nc.sync.dma_start(
                            out=out[0:NOFF, base + g4 * SUB : base + (g4 + 1) * SUB],
                            in_=stage[32 * g4 : 32 * g4 + NOFF, :],
                        )

    _split_sync_waits(nc)
    return nc


_PROGRAM = None


def _get_program():
    global _PROGRAM
    if _PROGRAM is None:
        _PROGRAM = build_program()
    return _PROGRAM


def _prep_inputs(in1, in2, conv_w, conv_b):
    """Build the 8 per-core input maps (bf16 layout prep on host)."""
    x1 = np.ascontiguousarray(np.asarray(in1, np.float32).reshape(C, D, H, W))
    x2 = np.ascontiguousarray(np.asarray(in2, np.float32).reshape(C, D, H, W))
    scale = 1.0 / np.sqrt(np.float32(C))
    Wk = (np.asarray(conv_w, np.float32) * scale).reshape(NOFF, NOFF, C)  # [o,k,c]

    wts = np.zeros((P96, NPASS * NOFF), np.float32)
    for dz in range(3):
        for dy in range(3):
            j = 3 * dz + dy
            for g in range(3):
                k = 9 * dz + 3 * dy + g
                wts[32 * g : 32 * g + C, j * NOFF : (j + 1) * NOFF] = Wk[:, k, :].T
    wts = wts.astype(ml_dtypes.bfloat16)

    bias128 = np.zeros((128, 1), np.float32)
    cb = np.asarray(conv_b, np.float32)
    for g4 in range(4):
        bias128[32 * g4 : 32 * g4 + NOFF, 0] = cb

    # Global zero-padded x2: pad plane/row/col index = global index + 1.
    x2p = np.zeros((C, D + 2, HP, WP), np.float32)
    x2p[:, 1 : D + 1, 1 : H + 1, 1 : W + 1] = x2

    in_maps = []
    for m in range(NCORES):
        slab = x2p[:, DLOC * m : DLOC * m + NSLAB]  # [C,10,66,66]
        flat = slab.reshape(C, -1)
        flat = np.concatenate([flat, np.zeros((C, 4), np.float32)], axis=1)
        # replica g = flat shifted by g (dx preshift), cut back to slab planes
        x2rep = np.stack(
            [flat[:, g : g + NSLAB * PLANE_F] for g in range(G)], axis=0
        )  # [3, C, 10*4356]
        x2rep = (
            x2rep.reshape(G * C, NSLAB, PLANE_F)
            .transpose(1, 0, 2)
            .astype(ml_dtypes.bfloat16)
        )  # [10, 96, 4356]

        x1c = x1[:, DLOC * m : DLOC * (m + 1)].reshape(C, -1)  # [C, 8*4096]
        x1rep = (
            np.tile(x1c, (G, 1))
            .reshape(P96, DLOC, H * W)
            .transpose(1, 0, 2)
            .astype(ml_dtypes.bfloat16)
        )  # [8, 96, 4096]

        in_maps.append(
            {
                "x1r": np.ascontiguousarray(x1rep),
                "x2r": np.ascontiguousarray(x2rep),
                "wts": np.ascontiguousarray(wts),
                "bias": bias128,
            }
        )
    return in_maps


def kernel(in1, in2, conv_w, conv_b):
    nc = _get_program()
    in_maps = _prep_inputs(in1, in2, conv_w, conv_b)
    res = run_bass_kernel_spmd(nc, in_maps, core_ids=list(range(NCORES)))
    outs = [r["out"].reshape(NOFF, DLOC, H, W) for r in res.results]
    full = np.concatenate(outs, axis=1)  # [27, 64, 64, 64]
    return full[None].astype(np.float32)  # [1, 27, 64, 64, 64]



# revision 5
# speedup vs baseline: 1.0733x; 1.0733x over previous
"""Trainium2 Bass kernel for nn_CorrTorch: 27-shift 3D correlation + 1x1x1 conv.

Math (B=1, C=32, D=H=W=64, NOFF=27):
  cv[(k,c), s] = x1[c,s] * pad(x2)[c, s + off_k] / sqrt(C)    (864 x 64^3)
  out[o, s]    = sum_{k,c} conv_w[o, k*32+c] * cv[(k,c), s] + conv_b[o]

Sharding: D axis split across 8 cores (8 planes each), 1-voxel halo baked into
the per-core x2 slab on the host. No collectives.

Per-core device strategy (v2):
  - 3 "replica groups" of 32 channels on partitions 0..95; group g holds data
    pre-shifted by dx=g along W (host-baked), so all 27 shifts reduce to
    9 product passes (dz,dy in {0,1,2}^2) per plane (provably minimal for any
    fixed-preshift replica scheme; no 4-group 7- or 8-cover of the 3^3 offset
    cube exists).  Each pass is column-split between the vector engine
    (rows 0..50 of the plane) and GPSIMD (rows 51..63) so both engines finish
    together -- the products are the bottleneck, everything else hides under
    them.
  - The 1x1 conv runs in the *flipped* matmul orientation: the cv chunk
    [96, 128] is the stationary operand and the tiny weight block [96, 27] is
    the moving operand, so each matmul streams only 27 columns (PSUM out tile
    [128 spatial, 27]).  A K=1 ones x bias matmul opens each accumulation
    group, folding the conv bias into PSUM init.
  - ScalarE evicts one PSUM bank (16 chunks = [128, 432]) at a time to bf16,
    and the DMA writes HBM in [spatial, 27] row-major layout; the host
    transposes to [27, D, H, W] and casts to fp32.
"""

import numpy as np
import ml_dtypes

import concourse.bass as bass
import concourse.mybir as mybir
import concourse.tile as tile
from concourse.bass_utils import run_bass_kernel_spmd

C = 32
D = 64
H = 64
W = 64
NOFF = 27
NCORES = 8
DLOC = D // NCORES          # 8 output planes per core
NSLAB = DLOC + 2            # 10 padded x2 planes per core
HP = H + 2                  # 66
WP = W + 2                  # 66
PLANE_F = HP * WP           # 4356 elements per padded plane per partition
G = 3                       # dx replica groups
P96 = G * C                 # 96 partitions used by products / contraction
NPASS = 9                   # (dz, dy) passes
TN = H * W                  # 4096 columns per cv tile (one full plane)
CHUNK = 128                 # spatial positions per flipped matmul
NCHUNK = TN // CHUNK        # 32 chunks per plane
BANKC = 16                  # chunks per PSUM bank ([128, 432] fp32)
VROWS = 51                  # plane rows computed on the vector engine
VCOLS = VROWS * W           # 3264 columns (25.5 chunks)

BF16 = mybir.dt.bfloat16
F32 = mybir.dt.float32

_wsplit_ctr = [0]


def _split_sync_waits(nc, max_waits=1):
    """Walrus in this container accepts at most one sync wait per instruction.
    Hoist excess waits onto NoOp instructions inserted just before, on the
    same engine (same-engine program order preserves the semantics)."""
    for fn in nc.m.functions:
        for bb in fn.blocks:
            new = []
            changed = False
            for ins in bb.instructions:
                si = ins.sync_info
                if si is not None and len(si.on_wait) > max_waits:
                    waits = list(si.on_wait)
                    excess, keep = waits[:-max_waits], waits[-max_waits:]
                    for i in range(0, len(excess), max_waits):
                        _wsplit_ctr[0] += 1
                        new.append(
                            mybir.InstNoOp(
                                name=f"wsplit-{_wsplit_ctr[0]}",
                                engine=ins.engine,
                                sync_info=mybir.SyncInfo(
                                    on_wait=excess[i : i + max_waits], on_update=[]
                                ),
                            )
                        )
                    ins.sync_info = mybir.SyncInfo(
                        on_wait=keep, on_update=list(si.on_update)
                    )
                    changed = True
                new.append(ins)
            if changed:
                bb.instructions = new


def build_program():
    nc = bass.Bass()

    x1r = nc.dram_tensor("x1r", [DLOC, P96, TN], BF16, kind="ExternalInput")
    x2r = nc.dram_tensor("x2r", [NSLAB, P96, PLANE_F], BF16, kind="ExternalInput")
    wts = nc.dram_tensor("wts", [P96, NPASS * NOFF], BF16, kind="ExternalInput")
    onesb = nc.dram_tensor("onesb", [1, CHUNK], BF16, kind="ExternalInput")
    biasw = nc.dram_tensor("biasw", [1, BANKC * NOFF], BF16, kind="ExternalInput")
    out = nc.dram_tensor("out", [DLOC * TN, NOFF], BF16, kind="ExternalOutput")

    with tile.TileContext(nc) as tc:
        with (
            tc.tile_pool(name="wt", bufs=1) as wt_pool,
            tc.tile_pool(name="x2", bufs=5) as x2_pool,
            tc.tile_pool(name="x1", bufs=3) as x1_pool,
            tc.tile_pool(name="cv", bufs=11) as cv_pool,
            tc.tile_pool(name="stage", bufs=4) as stage_pool,
            tc.tile_pool(name="psum", bufs=4, space="PSUM") as psum_pool,
        ):
            x2t = {}

            def load_x2_plane(p):
                t = x2_pool.tile([P96, HP, WP], BF16, tag="x2plane")
                nc.sync.dma_start(out=t[:], in_=x2r[p])
                x2t[p] = t

            # first compute pass (dz=0) needs only x1[0] + x2 slab 0
            x1t0 = x1_pool.tile([P96, TN], BF16, tag="x1plane")
            nc.sync.dma_start(out=x1t0[:], in_=x1r[0])
            load_x2_plane(0)
            wt_tile = wt_pool.tile([P96, NPASS * NOFF], BF16)
            nc.sync.dma_start(out=wt_tile[:], in_=wts[:])
            ones_tile = wt_pool.tile([1, CHUNK], BF16)
            nc.sync.dma_start(out=ones_tile[:], in_=onesb[:])
            biasw_tile = wt_pool.tile([1, BANKC * NOFF], BF16)
            nc.sync.dma_start(out=biasw_tile[:], in_=biasw[:])
            for p in range(1, 3):
                load_x2_plane(p)

            for d in range(DLOC):
                if d + 3 < NSLAB:
                    load_x2_plane(d + 3)
                if d == 0:
                    x1t = x1t0
                else:
                    x1t = x1_pool.tile([P96, TN], BF16, tag="x1plane")
                    nc.sync.dma_start(out=x1t[:], in_=x1r[d])

                psums = []
                for _ph in range(2):
                    ps = psum_pool.tile([128, BANKC * NOFF], F32, tag="ps")
                    psums.append(ps)
                # bank-wide bias init: psum bank = ones^T x (bias tiled x16).
                # start_tensor_calc zeroes the whole 2KB bank, so exactly one
                # start per bank.
                for ps in psums:
                    nc.tensor.matmul(
                        ps[:],
                        lhsT=ones_tile[0:1, :],
                        rhs=biasw_tile[0:1, :],
                        start=True,
                        stop=False,
                        skip_group_check=True,
                    )

                cvs = []
                for dz in range(3):
                    for dy in range(3):
                        j = 3 * dz + dy
                        cv = cv_pool.tile([P96, TN], BF16, tag="cv")
                        # vector engine: plane rows 0..VROWS-1
                        nc.vector.tensor_mul(
                            out=cv[:, 0:VCOLS],
                            in0=x1t[:, 0:VCOLS],
                            in1=x2t[d + dz][:, dy : dy + VROWS, 0:W],
                        )
                        # gpsimd: plane rows VROWS..63, in parallel
                        nc.gpsimd.tensor_mul(
                            out=cv[:, VCOLS:TN],
                            in0=x1t[:, VCOLS:TN],
                            in1=x2t[d + dz][:, dy + VROWS : dy + H, 0:W],
                        )
                        cvs.append(cv)
                        # flipped matmuls: stationary cv chunk, stream 27
                        # weight columns into psum [128 spatial, 27]
                        for ch in range(NCHUNK):
                            ps = psums[ch // BANKC]
                            col = (ch % BANKC) * NOFF
                            nc.tensor.matmul(
                                ps[:, col : col + NOFF],
                                lhsT=cv[:, ch * CHUNK : (ch + 1) * CHUNK],
                                rhs=wt_tile[:, j * NOFF : (j + 1) * NOFF],
                                start=False,
                                stop=(j == NPASS - 1 and ch % BANKC == BANKC - 1),
                                skip_group_check=True,
                            )

                for half in range(2):
                    stage = stage_pool.tile([128, BANKC * NOFF], BF16, tag="stage")
                    nc.scalar.activation(
                        stage[:],
                        psums[half][:],
                        mybir.ActivationFunctionType.Identity,
                    )
                    base = d * TN + half * (BANKC * CHUNK)
                    # out rows (base + 128*i + p), 27 contiguous bf16 each
                    nc.sync.dma_start(
                        out=out[base : base + BANKC * CHUNK, :],
                        in_=stage[:],
                    )

    _split_sync_waits(nc)
    return nc


_PROGRAM = None


def _get_program():
    global _PROGRAM
    if _PROGRAM is None:
        _PROGRAM = build_program()
    return _PROGRAM


def _prep_inputs(in1, in2, conv_w, conv_b):
    """Build the 8 per-core input maps (bf16 layout prep on host)."""
    x1 = np.ascontiguousarray(np.asarray(in1, np.float32).reshape(C, D, H, W))
    x2 = np.ascontiguousarray(np.asarray(in2, np.float32).reshape(C, D, H, W))
    scale = 1.0 / np.sqrt(np.float32(C))
    Wk = (np.asarray(conv_w, np.float32) * scale).reshape(NOFF, NOFF, C)  # [o,k,c]

    wts = np.zeros((P96, NPASS * NOFF), np.float32)
    for dz in range(3):
        for dy in range(3):
            j = 3 * dz + dy
            for g in range(3):
                k = 9 * dz + 3 * dy + g
                wts[32 * g : 32 * g + C, j * NOFF : (j + 1) * NOFF] = Wk[:, k, :].T
    wts = wts.astype(ml_dtypes.bfloat16)

    onesb = np.ones((1, CHUNK), np.float32).astype(ml_dtypes.bfloat16)
    biasw = np.tile(np.asarray(conv_b, np.float32)[None, :], (1, BANKC)).astype(
        ml_dtypes.bfloat16
    )  # [1, 432]

    # Global zero-padded x2: pad plane/row/col index = global index + 1.
    x2p = np.zeros((C, D + 2, HP, WP), np.float32)
    x2p[:, 1 : D + 1, 1 : H + 1, 1 : W + 1] = x2

    in_maps = []
    for m in range(NCORES):
        slab = x2p[:, DLOC * m : DLOC * m + NSLAB]  # [C,10,66,66]
        flat = slab.reshape(C, -1)
        flat = np.concatenate([flat, np.zeros((C, 4), np.float32)], axis=1)
        # replica g = flat shifted by g (dx preshift), cut back to slab planes
        x2rep = np.stack(
            [flat[:, g : g + NSLAB * PLANE_F] for g in range(G)], axis=0
        )  # [3, C, 10*4356]
        x2rep = (
            x2rep.reshape(P96, NSLAB, PLANE_F)
            .transpose(1, 0, 2)
            .astype(ml_dtypes.bfloat16)
        )  # [10, 96, 4356]

        x1c = x1[:, DLOC * m : DLOC * (m + 1)].reshape(C, -1)  # [C, 8*4096]
        x1rep = (
            np.tile(x1c, (G, 1))
            .reshape(P96, DLOC, TN)
            .transpose(1, 0, 2)
            .astype(ml_dtypes.bfloat16)
        )  # [8, 96, 4096]

        in_maps.append(
            {
                "x1r": np.ascontiguousarray(x1rep),
                "x2r": np.ascontiguousarray(x2rep),
                "wts": np.ascontiguousarray(wts),
                "onesb": onesb,
                "biasw": biasw,
            }
        )
    return in_maps


def kernel(in1, in2, conv_w, conv_b):
    nc = _get_program()
    in_maps = _prep_inputs(in1, in2, conv_w, conv_b)
    res = run_bass_kernel_spmd(nc, in_maps, core_ids=list(range(NCORES)))
    outs = []
    for r in res.results:
        # DMA wrote rows in (plane, half, partition, chunk) order; spatial
        # position within a half is chunk*128 + partition.
        arr = np.asarray(r["out"], np.float32).reshape(DLOC, 2, CHUNK, BANKC, NOFF)
        arr = arr.transpose(0, 1, 3, 2, 4).reshape(DLOC, H, W, NOFF)
        outs.append(arr)
    full = np.concatenate(outs, axis=0)  # [64, 64, 64, 27]
    return np.ascontiguousarray(full.transpose(3, 0, 1, 2))[None]  # [1,27,64,64,64]


# revision 9
# speedup vs baseline: 1.0782x; 1.0046x over previous
"""Trainium2 Bass kernel for nn_CorrTorch: 27-shift 3D correlation + 1x1x1 conv.

Math (B=1, C=32, D=H=W=64, NOFF=27):
  cv[(k,c), s] = x1[c,s] * pad(x2)[c, s + off_k] / sqrt(C)    (864 x 64^3)
  out[o, s]    = sum_{k,c} conv_w[o, k*32+c] * cv[(k,c), s] + conv_b[o]

Sharding: D axis split across 8 cores (8 planes each), 1-voxel halo baked into
the per-core x2 slab on the host. No collectives.

Per-core device strategy (v2):
  - 3 "replica groups" of 32 channels on partitions 0..95; group g holds data
    pre-shifted by dx=g along W (host-baked), so all 27 shifts reduce to
    9 product passes (dz,dy in {0,1,2}^2) per plane (provably minimal for any
    fixed-preshift replica scheme; no 4-group 7- or 8-cover of the 3^3 offset
    cube exists).  Each pass is column-split between the vector engine
    (rows 0..50 of the plane) and GPSIMD (rows 51..63) so both engines finish
    together -- the products are the bottleneck, everything else hides under
    them.
  - The 1x1 conv runs in the *flipped* matmul orientation: the cv chunk
    [96, 128] is the stationary operand and the tiny weight block [96, 27] is
    the moving operand, so each matmul streams only 27 columns (PSUM out tile
    [128 spatial, 27]).  A K=1 ones x bias matmul opens each accumulation
    group, folding the conv bias into PSUM init.
  - ScalarE evicts one PSUM bank (16 chunks = [128, 432]) at a time to bf16,
    and the DMA writes HBM in [spatial, 27] row-major layout; the host
    transposes to [27, D, H, W] and casts to fp32.
"""

import numpy as np
import ml_dtypes

import concourse.bass as bass
import concourse.mybir as mybir
import concourse.tile as tile
from concourse.bass_utils import run_bass_kernel_spmd

C = 32
D = 64
H = 64
W = 64
NOFF = 27
NCORES = 8
DLOC = D // NCORES          # 8 output planes per core
NSLAB = DLOC + 2            # 10 padded x2 planes per core
HP = H + 2                  # 66
WP = W + 2                  # 66
PLANE_F = HP * WP           # 4356 elements per padded plane per partition
G = 3                       # dx replica groups
P96 = G * C                 # 96 partitions used by products / contraction
NPASS = 9                   # (dz, dy) passes
TN = H * W                  # 4096 columns per cv tile (one full plane)
CHUNK = 128                 # spatial positions per flipped matmul
NCHUNK = TN // CHUNK        # 32 chunks per plane
BANKC = 16                  # chunks per PSUM bank ([128, 432] fp32)
VROWS = 51                  # plane rows computed on the vector engine
VCOLS = VROWS * W           # 3264 columns (25.5 chunks)
PROWS = H - VROWS           # 13 rows on gpsimd

BF16 = mybir.dt.bfloat16
F32 = mybir.dt.float32

_wsplit_ctr = [0]


def _split_sync_waits(nc, max_waits=1):
    """Walrus in this container accepts at most one sync wait per instruction.
    Hoist excess waits onto NoOp instructions inserted just before, on the
    same engine (same-engine program order preserves the semantics)."""
    for fn in nc.m.functions:
        for bb in fn.blocks:
            new = []
            changed = False
            for ins in bb.instructions:
                si = ins.sync_info
                if si is not None and len(si.on_wait) > max_waits:
                    waits = list(si.on_wait)
                    excess, keep = waits[:-max_waits], waits[-max_waits:]
                    for i in range(0, len(excess), max_waits):
                        _wsplit_ctr[0] += 1
                        new.append(
                            mybir.InstNoOp(
                                name=f"wsplit-{_wsplit_ctr[0]}",
                                engine=ins.engine,
                                sync_info=mybir.SyncInfo(
                                    on_wait=excess[i : i + max_waits], on_update=[]
                                ),
                            )
                        )
                    ins.sync_info = mybir.SyncInfo(
                        on_wait=keep, on_update=list(si.on_update)
                    )
                    changed = True
                new.append(ins)
            if changed:
                bb.instructions = new


def build_program():
    nc = bass.Bass()

    x1r = nc.dram_tensor("x1r", [DLOC, P96, TN], BF16, kind="ExternalInput")
    x2r = nc.dram_tensor("x2r", [NSLAB, P96, PLANE_F], BF16, kind="ExternalInput")
    wts = nc.dram_tensor("wts", [P96, NPASS * NOFF], BF16, kind="ExternalInput")
    onesb = nc.dram_tensor("onesb", [1, CHUNK], BF16, kind="ExternalInput")
    biasw = nc.dram_tensor("biasw", [1, BANKC * NOFF], BF16, kind="ExternalInput")
    out = nc.dram_tensor("out", [DLOC * TN, NOFF], BF16, kind="ExternalOutput")

    with tile.TileContext(nc) as tc:
        with (
            tc.tile_pool(name="wt", bufs=1) as wt_pool,
            tc.tile_pool(name="x2", bufs=5) as x2_pool,
            tc.tile_pool(name="x1", bufs=3) as x1_pool,
            tc.tile_pool(name="cv", bufs=11) as cv_pool,
            tc.tile_pool(name="stage", bufs=4) as stage_pool,
            tc.tile_pool(name="psum", bufs=4, space="PSUM") as psum_pool,
        ):
            x2t = {}

            def load_x2_plane(p):
                t = x2_pool.tile([P96, HP, WP], BF16, tag="x2plane")
                nc.sync.dma_start(out=t[:], in_=x2r[p])
                x2t[p] = t

            # first compute pass (dz=0) needs only x1[0] + x2 slab 0
            x1t0 = x1_pool.tile([P96, TN], BF16, tag="x1plane")
            nc.sync.dma_start(out=x1t0[:], in_=x1r[0])
            load_x2_plane(0)
            wt_tile = wt_pool.tile([P96, NPASS * NOFF], BF16)
            nc.sync.dma_start(out=wt_tile[:], in_=wts[:])
            ones_tile = wt_pool.tile([1, CHUNK], BF16)
            nc.sync.dma_start(out=ones_tile[:], in_=onesb[:])
            biasw_tile = wt_pool.tile([1, BANKC * NOFF], BF16)
            nc.sync.dma_start(out=biasw_tile[:], in_=biasw[:])
            for p in range(1, 3):
                load_x2_plane(p)

            for d in range(DLOC):
                if d + 3 < NSLAB:
                    load_x2_plane(d + 3)
                if d == 0:
                    x1t = x1t0
                else:
                    x1t = x1_pool.tile([P96, TN], BF16, tag="x1plane")
                    nc.sync.dma_start(out=x1t[:], in_=x1r[d])

                psums = []
                for _ph in range(2):
                    ps = psum_pool.tile([128, BANKC * NOFF], F32, tag="ps")
                    psums.append(ps)
                # bank-wide bias init: psum bank = ones^T x (bias tiled x16).
                # start_tensor_calc zeroes the whole 2KB bank, so exactly one
                # start per bank.
                for ps in psums:
                    nc.tensor.matmul(
                        ps[:],
                        lhsT=ones_tile[0:1, :],
                        rhs=biasw_tile[0:1, :],
                        start=True,
                        stop=False,
                        skip_group_check=True,
                    )

                cvs = []
                for dz in range(3):
                    for dy in range(3):
                        j = 3 * dz + dy
                        cv = cv_pool.tile([P96, TN], BF16, tag="cv")
                        # vector engine: plane rows 0..VROWS-1 (the very
                        # last pass is split so bank0's tail matmuls + evict
                        # + DMA can overlap the second piece)
                        if d == DLOC - 1 and j == NPASS - 1:
                            nc.vector.tensor_mul(
                                out=cv[:, 0:2048],
                                in0=x1t[:, 0:2048],
                                in1=x2t[d + dz][:, dy : dy + 32, 0:W],
                            )
                            nc.vector.tensor_mul(
                                out=cv[:, 2048:VCOLS],
                                in0=x1t[:, 2048:VCOLS],
                                in1=x2t[d + dz][:, dy + 32 : dy + VROWS, 0:W],
                            )
                        else:
                            nc.vector.tensor_mul(
                                out=cv[:, 0:VCOLS],
                                in0=x1t[:, 0:VCOLS],
                                in1=x2t[d + dz][:, dy : dy + VROWS, 0:W],
                            )
                        # gpsimd: plane rows VROWS..63, in parallel
                        nc.gpsimd.tensor_mul(
                            out=cv[:, VCOLS:TN],
                            in0=x1t[:, VCOLS:TN],
                            in1=x2t[d + dz][:, dy + VROWS : dy + H, 0:W],
                        )
                        cvs.append(cv)
                        # flipped matmuls: stationary cv chunk, stream 27
                        # weight columns into psum [128 spatial, 27]
                        for ch in range(NCHUNK):
                            ps = psums[ch // BANKC]
                            col = (ch % BANKC) * NOFF
                            nc.tensor.matmul(
                                ps[:, col : col + NOFF],
                                lhsT=cv[:, ch * CHUNK : (ch + 1) * CHUNK],
                                rhs=wt_tile[:, j * NOFF : (j + 1) * NOFF],
                                start=False,
                                stop=(j == NPASS - 1 and ch % BANKC == BANKC - 1),
                                skip_group_check=True,
                            )

                for half in range(2):
                    stage = stage_pool.tile([128, BANKC * NOFF], BF16, tag="stage")
                    nc.scalar.activation(
                        stage[:],
                        psums[half][:],
                        mybir.ActivationFunctionType.Identity,
                    )
                    base = d * TN + half * (BANKC * CHUNK)
                    # out rows (base + 128*i + p), 27 contiguous bf16 each
                    nc.sync.dma_start(
                        out=out[base : base + BANKC * CHUNK, :],
                        in_=stage[:],
                    )

    _split_sync_waits(nc)
    return nc


_PROGRAM = None


def _get_program():
    global _PROGRAM
    if _PROGRAM is None:
        _PROGRAM = build_program()
    return _PROGRAM


def _prep_inputs(in1, in2, conv_w, conv_b):
    """Build the 8 per-core input maps (bf16 layout prep on host)."""
    x1 = np.ascontiguousarray(np.asarray(in1, np.float32).reshape(C, D, H, W))
    x2 = np.ascontiguousarray(np.asarray(in2, np.float32).reshape(C, D, H, W))
    scale = 1.0 / np.sqrt(np.float32(C))
    Wk = (np.asarray(conv_w, np.float32) * scale).reshape(NOFF, NOFF, C)  # [o,k,c]

    wts = np.zeros((P96, NPASS * NOFF), np.float32)
    for dz in range(3):
        for dy in range(3):
            j = 3 * dz + dy
            for g in range(3):
                k = 9 * dz + 3 * dy + g
                wts[32 * g : 32 * g + C, j * NOFF : (j + 1) * NOFF] = Wk[:, k, :].T
    wts = wts.astype(ml_dtypes.bfloat16)

    onesb = np.ones((1, CHUNK), np.float32).astype(ml_dtypes.bfloat16)
    biasw = np.tile(np.asarray(conv_b, np.float32)[None, :], (1, BANKC)).astype(
        ml_dtypes.bfloat16
    )  # [1, 432]

    # Global zero-padded x2: pad plane/row/col index = global index + 1.
    x2p = np.zeros((C, D + 2, HP, WP), np.float32)
    x2p[:, 1 : D + 1, 1 : H + 1, 1 : W + 1] = x2

    in_maps = []
    for m in range(NCORES):
        slab = x2p[:, DLOC * m : DLOC * m + NSLAB]  # [C,10,66,66]
        flat = slab.reshape(C, -1)
        flat = np.concatenate([flat, np.zeros((C, 4), np.float32)], axis=1)
        # replica g = flat shifted by g (dx preshift), cut back to slab planes
        x2rep = np.stack(
            [flat[:, g : g + NSLAB * PLANE_F] for g in range(G)], axis=0
        )  # [3, C, 10*4356]
        x2rep = (
            x2rep.reshape(P96, NSLAB, PLANE_F)
            .transpose(1, 0, 2)
            .astype(ml_dtypes.bfloat16)
        )  # [10, 96, 4356]

        x1c = x1[:, DLOC * m : DLOC * (m + 1)].reshape(C, -1)  # [C, 8*4096]
        x1rep = (
            np.tile(x1c, (G, 1))
            .reshape(P96, DLOC, TN)
            .transpose(1, 0, 2)
            .astype(ml_dtypes.bfloat16)
        )  # [8, 96, 4096]

        in_maps.append(
            {
                "x1r": np.ascontiguousarray(x1rep),
                "x2r": np.ascontiguousarray(x2rep),
                "wts": np.ascontiguousarray(wts),
                "onesb": onesb,
                "biasw": biasw,
            }
        )
    return in_maps


def kernel(in1, in2, conv_w, conv_b):
    nc = _get_program()
    in_maps = _prep_inputs(in1, in2, conv_w, conv_b)
    res = run_bass_kernel_spmd(nc, in_maps, core_ids=list(range(NCORES)))
    outs = []
    for r in res.results:
        # DMA wrote rows in (plane, half, partition, chunk) order; spatial
        # position within a half is chunk*128 + partition.
        arr = np.asarray(r["out"], np.float32).reshape(DLOC, 2, CHUNK, BANKC, NOFF)
        arr = arr.transpose(0, 1, 3, 2, 4).reshape(DLOC, H, W, NOFF)
        outs.append(arr)
    full = np.concatenate(outs, axis=0)  # [64, 64, 64, 27]
    return np.ascontiguousarray(full.transpose(3, 0, 1, 2))[None]  # [1,27,64,64,64]


# revision 10
# speedup vs baseline: 1.0860x; 1.0073x over previous
"""Trainium2 Bass kernel for nn_CorrTorch: 27-shift 3D correlation + 1x1x1 conv.

Math (B=1, C=32, D=H=W=64, NOFF=27):
  cv[(k,c), s] = x1[c,s] * pad(x2)[c, s + off_k] / sqrt(C)    (864 x 64^3)
  out[o, s]    = sum_{k,c} conv_w[o, k*32+c] * cv[(k,c), s] + conv_b[o]

Sharding: D axis split across 8 cores (8 planes each), 1-voxel halo baked into
the per-core x2 slab on the host. No collectives.

Per-core device strategy (v2):
  - 3 "replica groups" of 32 channels on partitions 0..95; group g holds data
    pre-shifted by dx=g along W (host-baked), so all 27 shifts reduce to
    9 product passes (dz,dy in {0,1,2}^2) per plane (provably minimal for any
    fixed-preshift replica scheme; no 4-group 7- or 8-cover of the 3^3 offset
    cube exists).  Each pass is column-split between the vector engine
    (rows 0..50 of the plane) and GPSIMD (rows 51..63) so both engines finish
    together -- the products are the bottleneck, everything else hides under
    them.
  - The 1x1 conv runs in the *flipped* matmul orientation: the cv chunk
    [96, 128] is the stationary operand and the tiny weight block [96, 27] is
    the moving operand, so each matmul streams only 27 columns (PSUM out tile
    [128 spatial, 27]).  A K=1 ones x bias matmul opens each accumulation
    group, folding the conv bias into PSUM init.
  - ScalarE evicts one PSUM bank (16 chunks = [128, 432]) at a time to bf16,
    and the DMA writes HBM in [spatial, 27] row-major layout; the host
    transposes to [27, D, H, W] and casts to fp32.
"""

import numpy as np
import ml_dtypes

import concourse.bass as bass
import concourse.mybir as mybir
import concourse.tile as tile
from concourse.bass_utils import run_bass_kernel_spmd

C = 32
D = 64
H = 64
W = 64
NOFF = 27
NCORES = 8
DLOC = D // NCORES          # 8 output planes per core
NSLAB = DLOC + 2            # 10 padded x2 planes per core
HP = H + 2                  # 66
WP = W + 2                  # 66
PLANE_F = HP * WP           # 4356 elements per padded plane per partition
G = 3                       # dx replica groups
P96 = G * C                 # 96 partitions used by products / contraction
NPASS = 9                   # (dz, dy) passes
TN = H * W                  # 4096 columns per cv tile (one full plane)
CHUNK = 128                 # spatial positions per flipped matmul
NCHUNK = TN // CHUNK        # 32 chunks per plane
BANKC = 16                  # chunks per PSUM bank ([128, 432] fp32)
VROWS = 51                  # plane rows computed on the vector engine
VCOLS = VROWS * W           # 3264 columns (25.5 chunks)
PROWS = H - VROWS           # 13 rows on gpsimd

BF16 = mybir.dt.bfloat16
F32 = mybir.dt.float32

_wsplit_ctr = [0]


def _split_sync_waits(nc, max_waits=1):
    """Walrus in this container accepts at most one sync wait per instruction.
    Hoist excess waits onto NoOp instructions inserted just before, on the
    same engine (same-engine program order preserves the semantics)."""
    for fn in nc.m.functions:
        for bb in fn.blocks:
            new = []
            changed = False
            for ins in bb.instructions:
                si = ins.sync_info
                if si is not None and len(si.on_wait) > max_waits:
                    waits = list(si.on_wait)
                    excess, keep = waits[:-max_waits], waits[-max_waits:]
                    for i in range(0, len(excess), max_waits):
                        _wsplit_ctr[0] += 1
                        new.append(
                            mybir.InstNoOp(
                                name=f"wsplit-{_wsplit_ctr[0]}",
                                engine=ins.engine,
                                sync_info=mybir.SyncInfo(
                                    on_wait=excess[i : i + max_waits], on_update=[]
                                ),
                            )
                        )
                    ins.sync_info = mybir.SyncInfo(
                        on_wait=keep, on_update=list(si.on_update)
                    )
                    changed = True
                new.append(ins)
            if changed:
                bb.instructions = new


def build_program():
    nc = bass.Bass()

    x1r = nc.dram_tensor("x1r", [DLOC, P96, TN], BF16, kind="ExternalInput")
    x2r = nc.dram_tensor("x2r", [NSLAB, P96, PLANE_F], BF16, kind="ExternalInput")
    wts = nc.dram_tensor("wts", [P96, NPASS * NOFF], BF16, kind="ExternalInput")
    onesb = nc.dram_tensor("onesb", [1, CHUNK], BF16, kind="ExternalInput")
    biasw = nc.dram_tensor("biasw", [1, BANKC * NOFF], BF16, kind="ExternalInput")
    out = nc.dram_tensor("out", [DLOC * TN, NOFF], BF16, kind="ExternalOutput")

    with tile.TileContext(nc) as tc:
        with (
            tc.tile_pool(name="wt", bufs=1) as wt_pool,
            tc.tile_pool(name="x2", bufs=5) as x2_pool,
            tc.tile_pool(name="x1", bufs=3) as x1_pool,
            tc.tile_pool(name="cv", bufs=11) as cv_pool,
            tc.tile_pool(name="stage", bufs=4) as stage_pool,
            tc.tile_pool(name="psum", bufs=4, space="PSUM") as psum_pool,
        ):
            x2t = {}

            def load_x2_plane(p):
                t = x2_pool.tile([P96, HP, WP], BF16, tag="x2plane")
                nc.sync.dma_start(out=t[:], in_=x2r[p])
                x2t[p] = t

            # first compute pass (dz=0) needs only x1[0] cols 0:VCOLS and
            # x2 slab 0 rows 0:53 -- load those two pieces first so the vector
            # engine starts ~1us earlier, then backfill the gpsimd pieces.
            x1t0 = x1_pool.tile([P96, TN], BF16, tag="x1plane")
            t0 = x2_pool.tile([P96, HP, WP], BF16, tag="x2plane")
            nc.sync.dma_start(out=x1t0[:, 0:VCOLS], in_=x1r[0][:, 0:VCOLS])
            nc.sync.dma_start(
                out=t0[:, 0 : VROWS + 2, :],
                in_=x2r[0][:, 0 : (VROWS + 2) * WP],
            )
            nc.sync.dma_start(out=x1t0[:, VCOLS:TN], in_=x1r[0][:, VCOLS:TN])
            nc.sync.dma_start(
                out=t0[:, VROWS + 2 : HP, :],
                in_=x2r[0][:, (VROWS + 2) * WP :],
            )
            x2t[0] = t0
            wt_tile = wt_pool.tile([P96, NPASS * NOFF], BF16)
            nc.sync.dma_start(out=wt_tile[:], in_=wts[:])
            ones_tile = wt_pool.tile([1, CHUNK], BF16)
            nc.sync.dma_start(out=ones_tile[:], in_=onesb[:])
            biasw_tile = wt_pool.tile([1, BANKC * NOFF], BF16)
            nc.sync.dma_start(out=biasw_tile[:], in_=biasw[:])
            for p in range(1, 3):
                load_x2_plane(p)

            for d in range(DLOC):
                if d + 3 < NSLAB:
                    load_x2_plane(d + 3)
                if d == 0:
                    x1t = x1t0
                else:
                    x1t = x1_pool.tile([P96, TN], BF16, tag="x1plane")
                    nc.sync.dma_start(out=x1t[:], in_=x1r[d])

                psums = []
                for _ph in range(2):
                    ps = psum_pool.tile([128, BANKC * NOFF], F32, tag="ps")
                    psums.append(ps)
                # bank-wide bias init: psum bank = ones^T x (bias tiled x16).
                # start_tensor_calc zeroes the whole 2KB bank, so exactly one
                # start per bank.
                for ps in psums:
                    nc.tensor.matmul(
                        ps[:],
                        lhsT=ones_tile[0:1, :],
                        rhs=biasw_tile[0:1, :],
                        start=True,
                        stop=False,
                        skip_group_check=True,
                    )

                cvs = []
                for dz in range(3):
                    for dy in range(3):
                        j = 3 * dz + dy
                        cv = cv_pool.tile([P96, TN], BF16, tag="cv")
                        # vector engine: plane rows 0..VROWS-1 (the very
                        # last pass is split so bank0's tail matmuls + evict
                        # + DMA can overlap the second piece)
                        if d == DLOC - 1 and j == NPASS - 1:
                            nc.vector.tensor_mul(
                                out=cv[:, 0:2048],
                                in0=x1t[:, 0:2048],
                                in1=x2t[d + dz][:, dy : dy + 32, 0:W],
                            )
                            nc.vector.tensor_mul(
                                out=cv[:, 2048:VCOLS],
                                in0=x1t[:, 2048:VCOLS],
                                in1=x2t[d + dz][:, dy + 32 : dy + VROWS, 0:W],
                            )
                        else:
                            nc.vector.tensor_mul(
                                out=cv[:, 0:VCOLS],
                                in0=x1t[:, 0:VCOLS],
                                in1=x2t[d + dz][:, dy : dy + VROWS, 0:W],
                            )
                        # gpsimd: plane rows VROWS..63, in parallel
                        nc.gpsimd.tensor_mul(
                            out=cv[:, VCOLS:TN],
                            in0=x1t[:, VCOLS:TN],
                            in1=x2t[d + dz][:, dy + VROWS : dy + H, 0:W],
                        )
                        cvs.append(cv)
                        # flipped matmuls: stationary cv chunk, stream 27
                        # weight columns into psum [128 spatial, 27]
                        for ch in range(NCHUNK):
                            ps = psums[ch // BANKC]
                            col = (ch % BANKC) * NOFF
                            nc.tensor.matmul(
                                ps[:, col : col + NOFF],
                                lhsT=cv[:, ch * CHUNK : (ch + 1) * CHUNK],
                                rhs=wt_tile[:, j * NOFF : (j + 1) * NOFF],
                                start=False,
                                stop=(j == NPASS - 1 and ch % BANKC == BANKC - 1),
                                skip_group_check=True,
                            )

                for half in range(2):
                    stage = stage_pool.tile([128, BANKC * NOFF], BF16, tag="stage")
                    nc.scalar.activation(
                        stage[:],
                        psums[half][:],
                        mybir.ActivationFunctionType.Identity,
                    )
                    base = d * TN + half * (BANKC * CHUNK)
                    # out rows (base + 128*i + p), 27 contiguous bf16 each
                    nc.sync.dma_start(
                        out=out[base : base + BANKC * CHUNK, :],
                        in_=stage[:],
                    )

    _split_sync_waits(nc)
    return nc


_PROGRAM = None


def _get_program():
    global _PROGRAM
    if _PROGRAM is None:
        _PROGRAM = build_program()
    return _PROGRAM


def _prep_inputs(in1, in2, conv_w, conv_b):
    """Build the 8 per-core input maps (bf16 layout prep on host)."""
    x1 = np.ascontiguousarray(np.asarray(in1, np.float32).reshape(C, D, H, W))
    x2 = np.ascontiguousarray(np.asarray(in2, np.float32).reshape(C, D, H, W))
    scale = 1.0 / np.sqrt(np.float32(C))
    Wk = (np.asarray(conv_w, np.float32) * scale).reshape(NOFF, NOFF, C)  # [o,k,c]

    wts = np.zeros((P96, NPASS * NOFF), np.float32)
    for dz in range(3):
        for dy in range(3):
            j = 3 * dz + dy
            for g in range(3):
                k = 9 * dz + 3 * dy + g
                wts[32 * g : 32 * g + C, j * NOFF : (j + 1) * NOFF] = Wk[:, k, :].T
    wts = wts.astype(ml_dtypes.bfloat16)

    onesb = np.ones((1, CHUNK), np.float32).astype(ml_dtypes.bfloat16)
    biasw = np.tile(np.asarray(conv_b, np.float32)[None, :], (1, BANKC)).astype(
        ml_dtypes.bfloat16
    )  # [1, 432]

    # Global zero-padded x2: pad plane/row/col index = global index + 1.
    x2p = np.zeros((C, D + 2, HP, WP), np.float32)
    x2p[:, 1 : D + 1, 1 : H + 1, 1 : W + 1] = x2

    in_maps = []
    for m in range(NCORES):
        slab = x2p[:, DLOC * m : DLOC * m + NSLAB]  # [C,10,66,66]
        flat = slab.reshape(C, -1)
        flat = np.concatenate([flat, np.zeros((C, 4), np.float32)], axis=1)
        # replica g = flat shifted by g (dx preshift), cut back to slab planes
        x2rep = np.stack(
            [flat[:, g : g + NSLAB * PLANE_F] for g in range(G)], axis=0
        )  # [3, C, 10*4356]
        x2rep = (
            x2rep.reshape(P96, NSLAB, PLANE_F)
            .transpose(1, 0, 2)
            .astype(ml_dtypes.bfloat16)
        )  # [10, 96, 4356]

        x1c = x1[:, DLOC * m : DLOC * (m + 1)].reshape(C, -1)  # [C, 8*4096]
        x1rep = (
            np.tile(x1c, (G, 1))
            .reshape(P96, DLOC, TN)
            .transpose(1, 0, 2)
            .astype(ml_dtypes.bfloat16)
        )  # [8, 96, 4096]

        in_maps.append(
            {
                "x1r": np.ascontiguousarray(x1rep),
                "x2r": np.ascontiguousarray(x2rep),
                "wts": np.ascontiguousarray(wts),
                "onesb": onesb,
                "biasw": biasw,
            }
        )
    return in_maps


def kernel(in1, in2, conv_w, conv_b):
    nc = _get_program()
    in_maps = _prep_inputs(in1, in2, conv_w, conv_b)
    res = run_bass_kernel_spmd(nc, in_maps, core_ids=list(range(NCORES)))
    outs = []
    for r in res.results:
        # DMA wrote rows in (plane, half, partition, chunk) order; spatial
        # position within a half is chunk*128 + partition.
        arr = np.asarray(r["out"], np.float32).reshape(DLOC, 2, CHUNK, BANKC, NOFF)
        arr = arr.transpose(0, 1, 3, 2, 4).reshape(DLOC, H, W, NOFF)
        outs.append(arr)
    full = np.concatenate(outs, axis=0)  # [64, 64, 64, 27]
    return np.ascontiguousarray(full.transpose(3, 0, 1, 2))[None]  # [1,27,64,64,64]
